# revision 104
# baseline (speedup 1.0000x reference)
"""Multi-head attention on 8 Trainium2 NeuronCores (head-parallel).

Problem: Q,K,V [4096,512] fp32; Wq/Wk/Wv [8,512,64]; Wo [512,512].
  out = concat_h(softmax(QWq_h (KWk_h)^T / sqrt(64)) VWv_h) @ Wo

Sharding: one head per core. Each core computes its head end-to-end plus
its slice of the output projection (out_h @ Wo[64h:64h+64, :]); the host
sums the 8 partial [4096,512] outputs (fp16 partials).

Numerics (validated on real HW vs the fp32 reference on the grading
inputs: absmax_rel ~ 1.5e-2 vs the 2e-2 gate):
  P1  projections: x ~= f16(x) + e5m2(x - f16(x)); q = Xf16 @ Wf16
      (fp16 matmuls) + Xr8 @ W8 (one fp8e5m2 DoubleRow matmul per 2
      contraction tiles, half rate) restores ~2^-14 effective input
      precision. q,k evicted as fp16 hi + fp16 lo pairs (fp16 hi/lo
      keeps the main-pass cross terms ~8x smaller than bf16 hi/lo
      would, which is what lets the attention weights live in fp8e4m3
      range). fp8e4m3 copies of the q/k parts are also staged in the
      128-partition DoubleRow layout for the score cross matmuls
      (rows <64 paired with rows >=64 in subtile 0, subtile 1 zero).
      v is bf16-projected and evicted fp8 with a fp8 ones column (the
      ones column makes attn.V also produce the softmax denominator).
  P2  per 512-query chunk (era), stats for chunk c+1 interleaved:
      stats: fp16 single-term scores [n-tile, m] -> DVE rowmax (fp32);
        rowmax split fp16 hi + fp16 lo, DMA-scattered into rows 64/65
        of the q-side operand (scores are ~1e4, so one fp16 row would
        carry +-4 absolute error; the two-row split gets ~2e-3, which
        keeps exp(s - m) within fp8 range).
      main: K=66 fp16 matmul = hi*hi with rows 64/65 = (-1)*(mh, ml),
        so PSUM = scores - rowmax directly; fp8 DoubleRow matmul adds
        both cross terms (k_hi q_lo + k_lo q_hi) in one instruction.
        ACT exp evicts PSUM -> fp8e4m3 attn weights.
      attn.V: one fp8 DoubleRow matmul per 2 key-tiles accumulates
        outT [65, 512] in PSUM; row 64 is the softmax sum.
  P3  Wo (fp16): partial[n-tile,512] = outT^T @ wo scaled by 1/sum per
      query (the stats/sc PSUM banks are idle in the last era, so most
      of the output projection overlaps the final era's compute, with
      exp batched 1024-wide there).

The double scores computation exists because softmax needs the query
index on partitions (per-partition reduce) while the attn.V matmul
needs the key index on partitions.

Scheduling notes (cost-model makespan 258.3us -> 218.7us):
 - DVE is the steady-state pacer (the 128 rowmax reduces are ~153us of
   its ~187us busy); everything movable is kept off DVE in eras 0-6.
 - All stream DMAs (Q01, K ring-8, V) are emitted back-to-back on the
   SP queue so no dependent DMA's wait ever head-blocks a transfer;
   tiny relocation DMAs ride the Pool SWDGE queue, whose waits resolve
   against Pool's own casts.
 - lo8 residuals are written to fp8 directly by the DVE subtract (no
   Pool cast + staging); outT+sum share one eviction; v evictions are
   batched per 512-key block.
 - The last era splits its sum-row eviction, rsum gathers (SP+ACT
   queues), and reciprocal so the rsum->rcp->wo->DMA tail chain is
   short; P3's two wo evictions split across DVE and ACT.
 - Rowmax hi/lo rows scatter PER N-TILE as soon as that tile's stats
   finish (not batched at chunk end), so the last tile's scatter never
   queues behind six earlier DMAs on the era boundary (-6.6us).
"""

from contextlib import ExitStack

import numpy as np

N = 4096
DIM = 512
H = 8
D = 64
P = 128
CH = 512  # query columns per era (chunk)


def build_head_kernel(ctx, tc, outs, ins, n=N, dim=DIM, d=D):
    import concourse.bass as bass
    import concourse.mybir as mybir
    from concourse.bass import ts, ds

    nc = tc.nc
    f32 = mybir.dt.float32
    f16 = mybir.dt.float16
    bf16 = mybir.dt.bfloat16
    f8 = mybir.dt.float8e4
    f8e5 = mybir.dt.float8e5
    AF = mybir.ActivationFunctionType
    DR = mybir.MatmulPerfMode.DoubleRow

    KC = dim // P      # projection contraction chunks (4)
    NT = n // P        # 128-row tiles of n (= m tiles) (32)
    NCH = n // CH      # eras (8)
    NTC = CH // P      # n-tiles per era (4)
    PAIRS = (n // 512) // 2  # stats st_ps pairs per n-tile (4)
    GRP = NT // 2      # main groups per era, 2 m-tiles each (16)
    NB = n // 512      # 512-wide blocks (8)
    assert n % 1024 == 0 and dim % P == 0 and CH == 512

    qtf_d, qr8_d = ins["QTF"], ins["QR8"]
    ktf_d, kr8_d = ins["KTF"], ins["KR8"]
    vt_d = ins["VT"]
    wqkf_d, wqk8_d = ins["wqkf"], ins["wqk8"]
    wv_d, wo_d = ins["wv"], ins["wo"]
    out_d = outs["out"]

    singles = ctx.enter_context(tc.tile_pool(name="singles", bufs=1))

    # Persistent SBUF tensors.
    At_ev = singles.tile([d + 2, n], f16)  # q hi; rows 64/65 = mh/ml (even eras)
    At_od = singles.tile([d + 2, n], f16)  # odd-era copy (avoids scatter WAR)
    Bh = singles.tile([d + 2, n], f16)     # k hi; rows 64/65 = -1
    # fp8 DoubleRow cross operands: hardware requires 128-partition DR tiles,
    # so both cross terms ride in subtile 0 (rows 0..63 = hi/lo pair A, rows
    # 64..127 = pair B) and subtile 1 is zero.
    q8 = singles.tile([P, NB, 2, 512], f8)  # sub0: rows<64 q_lo8, >=64 q_hi8
    k8 = singles.tile([P, NT, 2, P], f8)    # sub0: rows<64 k_hi8, >=64 k_lo8
    qh8_t = singles.tile([d, n], f8)        # staging for the DMA relocations
    kl8_t = singles.tile([d, n], f8)
    # v tiles + ones column, padded to stride 80: the fp8 DoubleRow ldweights
    # requires the outer free step to be even and 16B-aligned (65 is not)
    VP = 80
    v_sb = singles.tile([P, NT, VP], f8)
    # attn_u @ v (rows 0..63) stacked with the softmax denominators (row 64)
    # so one ACT eviction covers both
    comb = singles.tile([d + 1, n], f16)
    rsum = singles.tile([P, NT], f16)      # sumexp gathered per n-tile
    rinv = singles.tile([P, NT], f32)
    # wq and wk ride in one tile each (hi + e5m2 residual): halves the
    # serialized weight-DMA preamble ahead of the K stream
    wf_sb = singles.tile([P, KC, 2 * d], f16)
    w8_sb = singles.tile([P, KC, 2 * d], f8e5)
    wqf_sb, wkf_sb = wf_sb[:, :, 0:d], wf_sb[:, :, d:2 * d]
    wq8_sb, wk8_sb = w8_sb[:, :, 0:d], w8_sb[:, :, d:2 * d]
    wv_sb = singles.tile([P, KC, d], bf16)
    wo_sb = singles.tile([d, dim], f16)

    def _load_w(w_sb, w_d):
        nc.sync.dma_start(out=w_sb, in_=w_d.rearrange("(c p) e -> p c e", p=P))

    nmax_pool = ctx.enter_context(tc.tile_pool(name="nmax_pool", bufs=7))

    nmax_tiles = {}

    def stats_item(c, g, pool):
        """One st_ps pair: n-tile scores vs 1024 keys + rowmax reduce."""
        j, p = divmod(g, PAIRS)
        gt = c * NTC + j  # global n-tile
        if p == 0:
            nmax_tiles[j] = nmax_pool.tile([P, PAIRS], f32, tag="nmax",
                                           name="nmax")
        st_ps = pool.tile([P, 1024], f32, tag="st_ps", name="st_ps")
        nc.tensor.matmul(st_ps[:, 0:512], lhsT=At_ev[0:d, ts(gt, P)],
                         rhs=Bh[0:d, ts(2 * p, 512)], start=True, stop=True)
        nc.tensor.matmul(st_ps[:, 512:1024], lhsT=At_ev[0:d, ts(gt, P)],
                         rhs=Bh[0:d, ts(2 * p + 1, 512)], start=True, stop=True)
        # NOTE: a fused tensor_tensor_reduce over the two 512-halves would
        # halve this in the cost model, but DVE can only read ONE PSUM
        # operand per instruction (walrus verifier NCC_IBVF027)
        nc.vector.reduce_max(nmax_tiles[j][:, p:p + 1], st_ps,
                             axis=mybir.AxisListType.X)
        if p == PAIRS - 1:
            if j == 0:  # first finished n-tile of this chunk: alloc bufs
                stats_item.cmax = nmax_pool.tile([P, NTC], f32, tag="cmax")
                stats_item.cmh = nmax_pool.tile([P, NTC], f16, tag="cmh")
                stats_item.cml = nmax_pool.tile([P, NTC], f16, tag="cml")
            cmax, cmh, cml = stats_item.cmax, stats_item.cmh, stats_item.cml
            nc.vector.reduce_max(cmax[:, j:j + 1], nmax_tiles[j],
                                 axis=mybir.AxisListType.X)
            # per-n-tile hi/lo split + scatter: each tile's rowmax rows ship
            # as soon as its own stats finish, so the LAST tile's scatter is
            # never queued behind 6 earlier DMAs at the era boundary
            nc.vector.tensor_copy(cmh[:, j:j + 1], cmax[:, j:j + 1])
            nc.vector.tensor_sub(cml[:, j:j + 1], cmax[:, j:j + 1],
                                 cmh[:, j:j + 1])
            At = At_ev if c % 2 == 0 else At_od
            nc.sync.dma_start(out=At[d:d + 1, ds(c * CH + j * P, P)],
                              in_=cmh[:, j:j + 1])
            nc.sync.dma_start(out=At[d + 1:d + 2, ds(c * CH + j * P, P)],
                              in_=cml[:, j:j + 1])

    # ---- P1: K/V projections + Q chunks 0-1, chunk-0 stats folded in.
    # Q chunks 2..7 are loaded and projected inside eras 0..5 (the input
    # DMA would otherwise gate era 0 by ~35us).
    pstream = ctx.enter_context(tc.tile_pool(name="pstream", bufs=3))

    def load_stream(t_d, tag, cols, nbs, dt=bf16):
        t = pstream.tile([P, KC, cols], dt, tag=tag, name=tag)
        nc.sync.dma_start(out=t, in_=t_d[:, nbs].rearrange("(c p) x -> p c x", p=P))
        return t

    def q_evict(nb):
        """Evict chunk-nb q from PSUM (held by q_evict.ps): fp16 hi + fp8
        DoubleRow-interleaved copies (+ At_od copy for odd chunks). The lo
        residual is written to fp8 directly by the DVE subtract; the hi8
        cast stages through kl8_t (dead after P1's k relocation)."""
        nbs = ds(nb * 512, 512)
        ps_q = q_evict.ps
        nc.scalar.copy(At_ev[0:d, nbs], ps_q)                    # hi = f16(q)
        nc.vector.tensor_sub(q8[0:d, nb, 0, :], ps_q, At_ev[0:d, nbs])  # lo8
        nc.gpsimd.tensor_copy(qh8_t[:, nbs], At_ev[0:d, nbs])    # hi8 staging
        # relocation rides the Pool queue: its wait is on Pool's own cast,
        # so it never head-blocks the SP stream queue
        nc.gpsimd.dma_start(out=q8[d:2 * d, nb, 0, :], in_=qh8_t[:, nbs])

    def at_od_copy(nb):
        """Odd-era main pass reads At_od (separate tile so the rowmax
        scatter never WARs the previous era's score matmul reads). Emitted
        well after q_evict(nb) so this DMA's wait never holds up the DMA
        queue's issue of later transfers (SP.SEQ is in-order)."""
        nbs = ds(nb * 512, 512)
        nc.sync.dma_start(out=At_od[0:d, nbs], in_=At_ev[0:d, nbs])

    def proj_mms(wf_sb, w8_sb, xf_t, xr_t, ps):
        """Projection matmuls: fp16 main term + e5m2 residual via DoubleRow.

        x ~= f16(x) + e5m2(x - f16(x)); w ~= f16(w); the residual term
        xr @ w8 restores ~2^-14 effective input precision at half rate."""
        thunks = []
        for kc in range(KC):
            thunks.append(lambda kc=kc: nc.tensor.matmul(
                ps, lhsT=wf_sb[:, kc, :], rhs=xf_t[:, kc, :],
                start=(kc == 0), stop=False))
        for i in range(KC // 2):
            thunks.append(lambda i=i: nc.tensor.matmul(
                ps, lhsT=w8_sb[:, 2 * i:2 * i + 2, :],
                rhs=xr_t[:, 2 * i:2 * i + 2, :],
                start=False, stop=(i == KC // 2 - 1), perf_mode=DR))
        return thunks

    def q_mms(nb, qtf_t, qr8_t, ps_q):
        return proj_mms(wqf_sb, wq8_sb, qtf_t, qr8_t, ps_q)

    pending = []  # chunk-0 stats thunks, emitted at spaced slots for overlap
    vt_blocks = []  # V stream tiles; blocks 2..7 are projected inside era 0

    def flush_one():
        if pending:
            pending.pop(0)()

    def v_proj(nb, ps_v):
        """Project V block nb into v_sb (4 m-tiles), one batched eviction."""
        for qtr in range(4):
            for kc in range(KC):
                nc.tensor.matmul(
                    ps_v[:, qtr, :],
                    lhsT=vt_blocks[nb][:, kc, qtr * P:(qtr + 1) * P],
                    rhs=wv_sb[:, kc, :],
                    start=(kc == 0), stop=(kc == KC - 1))
            flush_one()
        nc.scalar.copy(v_sb[:, 4 * nb:4 * nb + 4, 0:d], ps_v)

    # P1 PSUM: st_p1 (chunk-0 stats, 3 bufs = 6 banks) + phase pools <= 8
    with tc.tile_pool(name="st_p1", bufs=3, space="PSUM") as st_p1:
        # All stream DMAs are emitted up front, back-to-back, so no dependent
        # DMA's wait ever sits between them in the SP queue: weights, then Q
        # chunks 0-1, then all of K in a ring-8 (device order: Q01 -> K).
        # V rides the ACT HWDGE queue and so lands right after K finishes.
        nc.sync.dma_start(out=wf_sb,
                          in_=wqkf_d.rearrange("(c p) x -> p c x", p=P))
        nc.sync.dma_start(out=w8_sb,
                          in_=wqk8_d.rearrange("(c p) x -> p c x", p=P))
        # chunk-0 Q first (q-proj 0 gates chunk-0 stats); chunk 1 plus the
        # late-needed wv/wo loads queue behind K so K lands ~4.5us earlier
        q01 = [(load_stream(qtf_d, "qtf", 512, ds(0, 512), f16),
                load_stream(qr8_d, "qr8", 512, ds(0, 512), f8e5))]
        # K stream order: blocks 0-3 interleave ktf/kr8 (the chunk-0 stats
        # ladder consumes them in pair order), blocks 4-7 load all ktf
        # before kr8 so the last block's full projection lands earlier
        kt_parts, kr_parts = {}, {}

        def _load_ktf(nb):
            t = pstream.tile([P, KC, 512], f16, tag="kt", name="kt", bufs=8)
            nc.sync.dma_start(
                out=t, in_=ktf_d[:, ds(nb * 512, 512)].rearrange(
                    "(c p) x -> p c x", p=P))
            kt_parts[nb] = t

        def _load_kr8(nb):
            t = pstream.tile([P, KC, 512], f8e5, tag="kr", name="kr", bufs=8)
            nc.sync.dma_start(
                out=t, in_=kr8_d[:, ds(nb * 512, 512)].rearrange(
                    "(c p) x -> p c x", p=P))
            kr_parts[nb] = t

        for nb in range(4):
            _load_ktf(nb)
            _load_kr8(nb)
        for nb in range(4, NB):
            _load_ktf(nb)
        for nb in range(4, NB):
            _load_kr8(nb)
        kt_blocks = [(kt_parts[nb], kr_parts[nb]) for nb in range(NB)]
        q01.append((load_stream(qtf_d, "qtf", 512, ds(512, 512), f16),
                    load_stream(qr8_d, "qr8", 512, ds(512, 512), f8e5)))
        _load_w(wv_sb, wv_d)
        nc.sync.dma_start(out=wo_sb, in_=wo_d)
        # one-time constants on Pool (DVE is the steady-state pacer)
        nc.gpsimd.memset(Bh[d:d + 2, :], -1.0)
        nc.gpsimd.memset(v_sb[:, :, d + 1:], 0.0)
        nc.gpsimd.memset(v_sb[:, :, d:d + 1], 1.0)
        # only the DoubleRow subtile-1 halves need zeros; subtile 0 is
        # fully written by the lo8 subtracts + hi8 relocation DMAs
        nc.gpsimd.memset(q8[:, :, 1, :], 0.0)
        nc.gpsimd.memset(k8[:, :, 1, :], 0.0)

        with tc.tile_pool(name="pq_ps", bufs=2, space="PSUM") as pq_pool:
            q_evict.ps = pq_pool.tile([d, 512], f32)
            for mm in q_mms(0, q01[0][0], q01[0][1], q_evict.ps):
                mm()
            q_evict(0)

        with tc.tile_pool(name="pk_ps", bufs=2, space="PSUM") as pk_pool:
            for nb in range(NB):
                nbs = ds(nb * 512, 512)
                ktf_t, kr8_t = kt_blocks[nb]
                ps_k = pk_pool.tile([d, 512], f32)
                for mm in proj_mms(wkf_sb, wk8_sb, ktf_t, kr8_t, ps_k):
                    mm()
                nc.scalar.copy(Bh[0:d, nbs], ps_k)
                nc.vector.tensor_sub(kl8_t[:, nbs], ps_k, Bh[0:d, nbs])  # lo8
                for qtr in range(4):  # a 512-key block covers 4 m-tiles
                    mt = 4 * nb + qtr
                    nc.gpsimd.tensor_copy(k8[0:d, mt, 0, :],
                                          Bh[0:d, ts(mt, P)])
                # one batched relocation DMA per 512-key block (Pool queue:
                # the kl8_t wait resolves alongside Pool's own hi8 cast)
                nc.gpsimd.dma_start(out=k8[d:2 * d, 4 * nb:4 * nb + 4, 0, :],
                                    in_=kl8_t[:, nbs])
                if nb >= 2:
                    flush_one()
                    flush_one()
                if nb % 2 == 1:
                    # k-blocks 2p, 2p+1 (p = nb//2) are projected: queue the
                    # chunk-0 stats pairs that contract against them
                    p = nb // 2
                    for j in range(NTC):
                        pending.append(
                            lambda j=j, p=p: stats_item(0, j * PAIRS + p, st_p1))
        with tc.tile_pool(name="pv_ps", bufs=2, space="PSUM") as pv_pool:
            # V streams queue right behind K (the chunk-0 rowmax isn't ready
            # before the V tail anyway, so its scatters lose nothing by
            # sitting behind V in the SP queue)
            for nb in range(NB):
                vt_t = pstream.tile([P, KC, 512], bf16, tag="vt", name="vt",
                                    bufs=8)
                nc.sync.dma_start(
                    out=vt_t,
                    in_=vt_d[:, ds(nb * 512, 512)].rearrange("(c p) x -> p c x", p=P))
                vt_blocks.append(vt_t)
            while pending:
                flush_one()
            # chunk-1 projection, deferred here so its late DMA never
            # head-blocks the PE program during the k phase
            q_evict.ps = st_p1.tile([P, 1024], f32, tag="st_ps",
                                    name="q1ps")[0:d, 0:512]
            for mm in q_mms(1, q01[1][0], q01[1][1], q_evict.ps):
                mm()
            q_evict(1)
            at_od_copy(1)  # after all stream DMA issues (its wait is long)
            for nb in range(NB):
                ps_v = pv_pool.tile([P, 4, d], f32, name="ps_v")
                v_proj(nb, ps_v)

    # ---- P2: stats (chunk c+1) interleaved with main (chunk c) ----
    # PSUM banks in P2: st 4 + sc 3 + av 1 = 8
    with tc.tile_pool(name="st_ps_pool", bufs=2, space="PSUM") as st_pool, \
         tc.tile_pool(name="sc_ps_pool", bufs=3, space="PSUM") as sc_pool, \
         tc.tile_pool(name="av_ps_pool", bufs=1, space="PSUM") as av_pool, \
         tc.tile_pool(name="att_pool", bufs=5) as att_pool:

        def wo_pair(t, o_sb2, evict="dve", split_dma=False):
            """Output projection for n-tiles t, t+1 scaled by 1/sumexp.

            Runs only in the last era (which has no stats work); the 1/sum
            scaling is a per-partition scalar multiply, which both DVE
            (tensor_scalar) and ACT (activation scale) can do — the caller
            picks whichever engine has slack. split_dma issues one output
            DMA per tile (for the final pair, so the last transfer is not
            gated on both evictions)."""
            for i in range(2):
                wops = sc_pool.tile([P, 512], f32, tag="sc", name="wops")
                nc.tensor.matmul(wops, lhsT=comb[0:d, ts(t + i, P)], rhs=wo_sb,
                                 start=True, stop=True)
                eng = evict if evict != "mixed" else ("dve" if i == 0 else "act")
                if eng == "dve":
                    nc.vector.tensor_scalar_mul(
                        o_sb2[:, i, :], wops, rinv[:, t + i:t + i + 1])
                else:
                    nc.scalar.activation(
                        o_sb2[:, i, :], wops,
                        AF.Copy, scale=rinv[:, t + i:t + i + 1])
                if split_dma:
                    nc.sync.dma_start(out=out_d[ds((t + i) * P, P), :],
                                      in_=o_sb2[:, i, :])
            if not split_dma:
                nc.sync.dma_start(
                    out=out_d[ds(t * P, 2 * P), :].rearrange(
                        "(i p) x -> p i x", i=2),
                    in_=o_sb2)

        def era(c):
            """Main pass for chunk c; stats for chunk c+1 interleaved; DMA +
            projection of q chunk c+2 in the stats-idle second half."""
            At = At_ev if c % 2 == 0 else At_od
            cs = ds(c * CH, CH)
            r_hi66 = At[:, cs]        # [d+2, 512], rows 64/65 = mh/ml
            av_ps = av_pool.tile([VP, 512], f32, tag="av")
            att_fifo = []  # (att_tile, g) awaiting attn.V, deferred 2 groups
            qproj = None
            if c + 2 < NCH:
                qcs = ds((c + 2) * 512, 512)
                qtf_t = load_stream(qtf_d, "qtf", 512, qcs, f16)
                qr8_t = load_stream(qr8_d, "qr8", 512, qcs, f8e5)
                qproj = (qtf_t, qr8_t)

            def emit_av(att_t, g):
                nc.tensor.matmul(av_ps, lhsT=v_sb[:, 2 * g:2 * g + 2, :],
                                 rhs=att_t, start=(g == 0), stop=(g == GRP - 1),
                                 perf_mode=DR)

            qproj_mms = []
            for g in range(GRP):
                # 2 stats items per group over the first half of the era, so
                # the rowmax scatter completes with half an era of slack
                # before era c+1's first score matmul reads it
                if c + 1 < NCH:
                    for k in (2 * g, 2 * g + 1):
                        if k < NTC * PAIRS:
                            stats_item(c + 1, k, st_pool)

                # q chunk c+2 projection in the stats-idle groups 8..14,
                # using a stats PSUM slot
                if qproj is not None and g == 8:
                    q_evict.ps = st_pool.tile([P, 1024], f32, tag="st_ps",
                                              name="qp")[0:d, 0:512]
                    qproj_mms = q_mms(c + 2, qproj[0], qproj[1], q_evict.ps)
                if qproj_mms:
                    qproj_mms.pop(0)()
                    if qproj_mms:
                        qproj_mms.pop(0)()
                    if not qproj_mms:
                        q_evict(c + 2)
                att_t = att_pool.tile([P, 2, 512], f8, tag="att")
                if c == NCH - 1:
                    # no stats in the last era: score pairs use the free st
                    # banks and exp runs 1024-wide (amortizes ACT's fixed
                    # SBUF/PSUM access cost on the critical tail)
                    scp = st_pool.tile([P, 1024], f32, tag="st_ps", name="scp")
                    for half in range(2):
                        mt = ts(2 * g + half, P)
                        sc_ps = scp[:, half * 512:(half + 1) * 512]
                        nc.tensor.matmul(sc_ps, lhsT=Bh[:, mt], rhs=r_hi66,
                                         start=True, stop=False)
                        nc.tensor.matmul(sc_ps, lhsT=k8[:, 2 * g + half],
                                         rhs=q8[:, c], start=False, stop=True,
                                         perf_mode=DR)
                    nc.scalar.activation(
                        att_t, scp.rearrange("p (a x) -> p a x", a=2), AF.Exp)
                else:
                    for half in range(2):
                        mt = ts(2 * g + half, P)
                        sc_ps = sc_pool.tile([P, 512], f32, tag="sc")
                        # hi*hi with the two rowmax-subtraction rows (K=66)
                        nc.tensor.matmul(sc_ps, lhsT=Bh[:, mt], rhs=r_hi66,
                                         start=True, stop=False)
                        # both cross terms in one fp8 DoubleRow matmul
                        nc.tensor.matmul(sc_ps, lhsT=k8[:, 2 * g + half],
                                         rhs=q8[:, c], start=False, stop=True,
                                         perf_mode=DR)
                        nc.scalar.activation(att_t[:, half, :], sc_ps, AF.Exp)
                # defer attn.V one group so the exp it reads is done
                att_fifo.append((att_t, g))
                if len(att_fifo) > 4:
                    emit_av(*att_fifo.pop(0))
                if c == NCH - 1 and 2 * g < NT - NTC:
                    # the sc pool is idle in the last era: it carries earlier
                    # chunks' output projection, overlapped with compute
                    o_sb2 = att_pool.tile([P, 2, dim], f16, tag="o_early",
                                          name="o_early")
                    wo_pair(2 * g, o_sb2)
            for item in att_fifo:
                emit_av(*item)
            # evict attn_u @ v together with the sumexp row (one fp16 copy),
            # then gather the per-n-tile denominators (batched). In the last
            # era the sum row goes first in its own copy: it gates the
            # rsum -> rcp -> wo tail chain.
            if c == NCH - 1:
                nc.scalar.copy(comb[d:d + 1, cs], av_ps[d:d + 1, :])
                nc.scalar.copy(comb[0:d, cs], av_ps[0:d, :])
            else:
                nc.scalar.copy(comb[:, cs], av_ps[0:d + 1, :])
            for jj in range(NTC):
                # in the last era the gather is on the rcp -> wo critical
                # tail: split it across the SP and (now idle) ACT queues
                q = nc.scalar if (c == NCH - 1 and jj % 2) else nc.sync
                q.dma_start(out=rsum[:, c * NTC + jj:c * NTC + jj + 1],
                            in_=comb[d:d + 1, ds(c * CH + jj * P, P)])
            if c == NCH - 1:
                # split so the first P3 wo pair unblocks after 2 gathers
                nc.vector.reciprocal(rinv[:, ds(c * NTC, 2)],
                                     rsum[:, ds(c * NTC, 2)])
                nc.vector.reciprocal(rinv[:, ds(c * NTC + 2, 2)],
                                     rsum[:, ds(c * NTC + 2, 2)])
            else:
                nc.vector.reciprocal(rinv[:, ds(c * NTC, NTC)],
                                     rsum[:, ds(c * NTC, NTC)])
            if c + 2 < NCH and (c + 2) % 2 == 1:
                at_od_copy(c + 2)  # its data is ready; emitted era-end so its
                # wait never blocks this era's scatter/stream DMA issues

        for c in range(NCH):
            era(c)

        # ---- P3 tail: last chunk's output projection (evictions split
        # across DVE and the now-idle ACT) ----
        for i, t in enumerate(range(NT - NTC, NT, 2)):
            o_sb2 = att_pool.tile([P, 2, dim], f16, tag="o_early", name="o_early")
            wo_pair(t, o_sb2, evict="mixed")


def _f16_res(x):
    """Split fp32 array into fp16 main + e5m2 residual with x ~= f + r."""
    import ml_dtypes

    f = x.astype(np.float16)
    r = (x - f.astype(np.float32)).astype(ml_dtypes.float8_e5m2)
    return np.ascontiguousarray(f), np.ascontiguousarray(r)


def make_in_maps(Q, K, V, Wq, Wk, Wv, Wo):
    """Host-side sharding: transpose activations, slice weights per head."""
    import ml_dtypes

    scale = 1.0 / np.sqrt(Wq.shape[-1])
    QTF, QR8 = _f16_res(np.ascontiguousarray(Q.T.astype(np.float32)))
    KTF, KR8 = _f16_res(np.ascontiguousarray(K.T.astype(np.float32)))
    VT = np.ascontiguousarray(V.T.astype(np.float32)).astype(ml_dtypes.bfloat16)
    d = Wq.shape[-1]
    in_maps = []
    for h in range(Wq.shape[0]):
        wq = Wq[h].astype(np.float32) * scale
        wk = Wk[h].astype(np.float32)
        wqk = np.concatenate([wq, wk], axis=1)  # [dim, 2*d]
        in_maps.append({
            "QTF": QTF, "QR8": QR8, "KTF": KTF, "KR8": KR8, "VT": VT,
            "wqkf": np.ascontiguousarray(wqk.astype(np.float16)),
            "wqk8": np.ascontiguousarray(wqk.astype(ml_dtypes.float8_e5m2)),
            "wv": np.ascontiguousarray(Wv[h].astype(ml_dtypes.bfloat16)),
            "wo": np.ascontiguousarray(
                Wo[h * d:(h + 1) * d, :].astype(np.float16)),
        })
    return in_maps


_CACHE = {}


def _build_and_compile(n=N, dim=DIM, d=D, num_cores=H, repeats=1):
    import concourse.bass as bass
    import concourse.mybir as mybir
    import concourse.tile as tile
    from concourse import bacc

    key = (n, dim, d, num_cores, repeats)
    if key in _CACHE:
        return _CACHE[key]
    nc = bacc.Bacc("TRN2", target_bir_lowering=False, debug=False,
                   num_devices=num_cores)
    f32 = mybir.dt.float32
    f16 = mybir.dt.float16
    bf16 = mybir.dt.bfloat16
    f8e5 = mybir.dt.float8e5
    ins = {}
    for name, dt in (("QTF", f16), ("KTF", f16), ("QR8", f8e5), ("KR8", f8e5),
                     ("VT", bf16)):
        ins[name] = nc.dram_tensor(name, [dim, n], dt, kind="ExternalInput").ap()
    ins["wqkf"] = nc.dram_tensor("wqkf", [dim, 2 * d], f16,
                                 kind="ExternalInput").ap()
    ins["wqk8"] = nc.dram_tensor("wqk8", [dim, 2 * d], f8e5,
                                 kind="ExternalInput").ap()
    ins["wv"] = nc.dram_tensor("wv", [dim, d], bf16, kind="ExternalInput").ap()
    ins["wo"] = nc.dram_tensor("wo", [d, dim], f16, kind="ExternalInput").ap()
    outs = {"out": nc.dram_tensor("out", [n, dim], f16, kind="ExternalOutput").ap()}
    with tile.TileContext(nc) as tc:
        for _rep in range(repeats):
            with ExitStack() as ctx:
                build_head_kernel(ctx, tc, outs, ins, n=n, dim=dim, d=d)
    nc.compile()
    _CACHE[key] = nc
    return nc


def run_on_hw(in_maps, trace=False, **kwargs):
    from concourse.bass_utils import run_bass_kernel_spmd

    nc = _build_and_compile(num_cores=len(in_maps))
    return run_bass_kernel_spmd(nc, in_maps, core_ids=list(range(len(in_maps))),
                                trace=trace, **kwargs)


def kernel(Q, K, V, Wq, Wk, Wv, Wo):
    in_maps = make_in_maps(np.asarray(Q), np.asarray(K), np.asarray(V),
                           np.asarray(Wq), np.asarray(Wk), np.asarray(Wv),
                           np.asarray(Wo))
    res = run_on_hw(in_maps)
    out = np.zeros((N, DIM), dtype=np.float64)
    for r in res.results:
        out += r["out"].astype(np.float64)
    return out.astype(np.float32)


if __name__ == "__main__":
    rng = np.random.default_rng(0)
    inputs = {
        "Q": rng.standard_normal((N, DIM), dtype=np.float32),
        "K": rng.standard_normal((N, DIM), dtype=np.float32),
        "V": rng.standard_normal((N, DIM), dtype=np.float32),
        "Wq": rng.random((H, DIM, D), dtype=np.float32),
        "Wk": rng.random((H, DIM, D), dtype=np.float32),
        "Wv": rng.random((H, DIM, D), dtype=np.float32),
        "Wo": rng.random((DIM, DIM), dtype=np.float32),
    }
    out = kernel(**inputs)
    print(out.shape, out.dtype, np.abs(out).max())

